# revision 34
# baseline (speedup 1.0000x reference)
"""Trainium2 Bass kernel for nn_DecoderBlock (upsample + skip-fusion + LN + Mamba).

Self-contained: hardcodes all shapes. Shards batch B=32 across 8 NeuronCores
(4 batches/core). Device layout is channels-first ([feature partitions, time]);
the host pre-transposes activations and pre-composes weights:
  * ConvTranspose1d(k=4,s=2,p=1) + channel-concat + 1x1 fusion conv
    -> 5 plain 512x512 matmuls (even/odd time phases + skip term)
  * LayerNorm mean removed exactly by column-centering those matrices;
    gamma folded into in_proj; var via ones-matmul on TensorE
  * depthwise conv k=4 -> 4 diagonal-matrix matmuls accumulated in PSUM
  * selective scan: decay exp on ScalarE, recurrence via tensor_tensor_scan,
    state-readout reduction over S on TensorE (identity-matmul PSUM accum)
"""
import numpy as np
import ml_dtypes

BF16 = ml_dtypes.bfloat16

D = 512        # d_model
DI = 1024      # d_inner
S = 16         # d_state
DTR = 32       # dt_rank
BTOT = 32      # total batch
TL = 512       # low-res time
T = 1024       # full time
NCORES = 8
BL = BTOT // NCORES   # batches per core

# ---- tuning knobs ----
BT_ON_GPS = 2          # b-term TTs on GPSIMD per s-group (of 8)
PROD_ON_GPS = 2        # readout TTs on GPSIMD per s-group (of 8)
DECAY_BF16 = True      # decay tile dtype (validated: bf16 costs ~0 accuracy)

_BUILT = None  # cached nc


def _host_prep(inputs):
    f32 = np.float32
    x = np.asarray(inputs["x"], f32)
    skip = np.asarray(inputs["skip"], f32)
    up_w = np.asarray(inputs["up_w"], f32)
    up_b = np.asarray(inputs["up_b"], f32)
    fus_w = np.asarray(inputs["fus_w"], f32)
    fus_b = np.asarray(inputs["fus_b"], f32)
    ln_g = np.asarray(inputs["ln_g"], f32)
    ln_b = np.asarray(inputs["ln_b"], f32)
    in_w = np.asarray(inputs["in_w"], f32)
    conv_w = np.asarray(inputs["conv_w"], f32)
    conv_b = np.asarray(inputs["conv_b"], f32)
    xproj_w = np.asarray(inputs["xproj_w"], f32)
    dt_w = np.asarray(inputs["dt_w"], f32)
    dt_b = np.asarray(inputs["dt_b"], f32)
    A_log = np.asarray(inputs["A_log"], f32)
    Dp = np.asarray(inputs["Dp"], f32)
    out_w = np.asarray(inputs["out_w"], f32)

    wt = np.swapaxes(up_w[:, :, ::-1], 0, 1)          # (out,in,k)
    fw_x, fw_s = fus_w[:, :D], fus_w[:, D:]
    M_e0 = fw_x @ wt[:, :, 0]
    M_e1 = fw_x @ wt[:, :, 2]
    M_o0 = fw_x @ wt[:, :, 1]
    M_o1 = fw_x @ wt[:, :, 3]
    fb = fw_x @ up_b + fus_b

    def center(M):
        return M - M.mean(axis=0, keepdims=True)

    mats = [center(m) for m in (M_e0, M_e1, M_o0, M_o1, fw_s)]
    fbc = fb - fb.mean()

    in_w_g = in_w * ln_g[None, :]
    c0 = in_w @ ln_b                                   # (2DI,)
    A = -np.exp(A_log[0, :]).astype(np.float64)        # (S,) rows identical
    assert np.abs(A_log - A_log[0:1, :]).max() == 0.0

    # --- device weight arrays (sliced-contiguous layouts) ---
    # w_front[kp, od, mat, ki, m128]
    WT = np.stack([m.T.reshape(4, 128, 4, 128) for m in mats])   # (5,ki,kp,od,m)
    w_front = WT.transpose(2, 3, 0, 1, 4).astype(BF16).copy()    # (128,od,5,ki,128)
    # w_in[kp, mt, ki, m128]
    w_in = in_w_g.T.reshape(4, 128, 16, 128).transpose(1, 2, 0, 3).astype(BF16).copy()
    w4 = conv_w[:, 0, :]                                         # (DI,4)
    w_conv = np.zeros((128, 8, 4, 128), f32)
    for dtile in range(8):
        for k in range(4):
            np.fill_diagonal(w_conv[:, dtile, k, :], w4[dtile * 128:(dtile + 1) * 128, k])
    w_conv = w_conv.astype(BF16)
    w_xp = xproj_w.T.reshape(8, 128, 64).transpose(1, 0, 2).astype(BF16).copy()  # (128,8,64)
    w_dt = np.zeros((128, DI), f32)
    w_dt[:DTR, :] = dt_w.T
    w_dt = w_dt.astype(BF16)
    # w_out[kp, od, ki, m128]
    w_out = out_w.T.reshape(8, 128, 4, 128).transpose(1, 2, 0, 3).astype(BF16).copy()
    ident = np.eye(128, dtype=f32).astype(BF16)
    ones = np.ones((128, 1), f32).astype(BF16)

    # biases packed [128, ncols]: fbc(4) c0x(8) c0z(8) conv_b(8) dt_b(8) Dp(8) eps(1)
    bias = np.zeros((128, 45), f32)
    bias[:, 0:4] = fbc.reshape(4, 128).T
    bias[:, 4:12] = c0[:DI].reshape(8, 128).T
    bias[:, 12:20] = c0[DI:].reshape(8, 128).T
    bias[:, 20:28] = conv_b.reshape(8, 128).T
    bias[:, 28:36] = dt_b.reshape(8, 128).T
    bias[:, 36:44] = Dp.reshape(8, 128).T
    bias[:, 44] = 1e-5

    # activations per core
    xs_ = x.transpose(0, 2, 1)                         # (B, D, TL)
    xpad = np.zeros((BTOT, D, TL + 2), f32)
    xpad[:, :, 1:TL + 1] = xs_
    xpad = xpad.astype(BF16)
    skT = skip.transpose(0, 2, 1)                      # (B, D, T)
    sk_e = skT[:, :, 0::2].astype(BF16).copy()
    sk_o = skT[:, :, 1::2].astype(BF16).copy()

    per_core = []
    for c in range(NCORES):
        sl = slice(c * BL, (c + 1) * BL)
        per_core.append(dict(
            xpad=np.ascontiguousarray(xpad[sl]),
            sk_e=np.ascontiguousarray(sk_e[sl]),
            sk_o=np.ascontiguousarray(sk_o[sl]),
        ))
    weights = dict(w_front=w_front, w_in=w_in, w_conv=w_conv, w_xp=w_xp,
                   w_dt=w_dt, w_out=w_out, ident=ident, ones=ones, bias=bias)
    return per_core, weights, A


def _build(A):
    import concourse.mybir as mybir
    import concourse.tile as tile
    from concourse import bacc
    from contextlib import ExitStack

    f32 = mybir.dt.float32
    bf16 = mybir.dt.bfloat16
    OP = mybir.AluOpType
    AF = mybir.ActivationFunctionType

    nc = bacc.Bacc("TRN2", target_bir_lowering=False, debug=False,
                   num_devices=NCORES)
    d_xpad = nc.dram_tensor("xpad", [BL, D, TL + 2], bf16, kind="ExternalInput")
    d_sk_e = nc.dram_tensor("sk_e", [BL, D, TL], bf16, kind="ExternalInput")
    d_sk_o = nc.dram_tensor("sk_o", [BL, D, TL], bf16, kind="ExternalInput")
    d_wf = nc.dram_tensor("w_front", [128, 4, 5, 4, 128], bf16, kind="ExternalInput")
    d_win = nc.dram_tensor("w_in", [128, 16, 4, 128], bf16, kind="ExternalInput")
    d_wcv = nc.dram_tensor("w_conv", [128, 8, 4, 128], bf16, kind="ExternalInput")
    d_wxp = nc.dram_tensor("w_xp", [128, 8, 64], bf16, kind="ExternalInput")
    d_wdt = nc.dram_tensor("w_dt", [128, DI], bf16, kind="ExternalInput")
    d_wout = nc.dram_tensor("w_out", [128, 4, 8, 128], bf16, kind="ExternalInput")
    d_id = nc.dram_tensor("ident", [128, 128], bf16, kind="ExternalInput")
    d_ones = nc.dram_tensor("ones", [128, 1], bf16, kind="ExternalInput")
    d_bias = nc.dram_tensor("bias", [128, 45], f32, kind="ExternalInput")
    d_out = nc.dram_tensor("outT", [BL, D, T], f32, kind="ExternalOutput")

    DEC_DT = bf16 if DECAY_BF16 else f32

    with tile.TileContext(nc) as tc:
        with ExitStack() as es:
            def pool(name, bufs, space="SBUF"):
                return es.enter_context(tc.tile_pool(name=name, bufs=bufs, space=space))
            cpool = pool("const", 1)
            wpool = pool("wstream", 2)
            ipool = pool("inp", 4)
            fpool = pool("fused", 5)
            sqpool = pool("sq", 5)
            fnpool = pool("fn", 5)
            xinpool = pool("xin", 3)
            xdpool = pool("xdbl", 2)
            reppool = pool("rep", 8)
            rrpool = pool("rrep", 2)
            bigpool = pool("big", 8)     # xs / dt resident per b
            upool = pool("u", 2)
            apool = pool("a", 3)
            btpool = pool("bt", 3)
            hpool = pool("h", 3)
            prpool = pool("prod", 2)
            ypool = pool("ysb", 9)
            zpool = pool("z", 3)
            spool = pool("small", 2)
            opool = pool("outs", 2)
            dpool = pool("dram", 3, "DRAM")
            pmm = pool("pm", 2, "PSUM")
            pym = pool("py", 2, "PSUM")
            psm = pool("ps", 1, "PSUM")

            # ---------- small constants ----------
            wxp = cpool.tile([128, 8, 64], bf16)
            nc.sync.dma_start(wxp[:], d_wxp[:])
            wdt = cpool.tile([128, DI], bf16)
            nc.sync.dma_start(wdt[:], d_wdt[:])
            idt = cpool.tile([128, 128], bf16)
            nc.sync.dma_start(idt[:], d_id[:])
            onesb = cpool.tile([128, 1], bf16)
            nc.sync.dma_start(onesb[:], d_ones[:])
            bias = cpool.tile([128, 45], f32)
            nc.sync.dma_start(bias[:], d_bias[:])

            def bias_col(c):
                return bias[:, c:c + 1]

            for b in range(BL):
                # ---------- load inputs ----------
                xp = []
                for ki in range(4):
                    t_ = ipool.tile([128, TL + 2], bf16, tag="xp", name="xp")
                    nc.sync.dma_start(t_[:], d_xpad[b, ki * 128:(ki + 1) * 128, :])
                    xp.append(t_)
                ske, sko = [], []
                for ki in range(4):
                    te = ipool.tile([128, TL], bf16, tag="ske", name="ske")
                    nc.sync.dma_start(te[:], d_sk_e[b, ki * 128:(ki + 1) * 128, :])
                    ske.append(te)
                    to = ipool.tile([128, TL], bf16, tag="sko", name="sko")
                    nc.sync.dma_start(to[:], d_sk_o[b, ki * 128:(ki + 1) * 128, :])
                    sko.append(to)

                # ---------- front end ----------
                fused = []   # per od: [128, 512, 2] bf16 (t-interleaved)
                sqs = []
                for od in range(4):
                    wf = wpool.tile([128, 5, 4, 128], bf16, tag="wf", name="wf")
                    nc.sync.dma_start(wf[:], d_wf[:, od])
                    pe = pmm.tile([128, 512], f32, tag="pm", name="pe")
                    po = pmm.tile([128, 512], f32, tag="pm", name="po")
                    n = 0
                    for ki in range(4):
                        nc.tensor.matmul(pe[:], wf[:, 0, ki, :], xp[ki][:, 0:TL],
                                         start=(n == 0), stop=False); n += 1
                        nc.tensor.matmul(pe[:], wf[:, 1, ki, :], xp[ki][:, 1:TL + 1],
                                         start=False, stop=False); n += 1
                        nc.tensor.matmul(pe[:], wf[:, 4, ki, :], ske[ki][:],
                                         start=False, stop=(n == 11)); n += 1
                    n = 0
                    for ki in range(4):
                        nc.tensor.matmul(po[:], wf[:, 2, ki, :], xp[ki][:, 1:TL + 1],
                                         start=(n == 0), stop=False); n += 1
                        nc.tensor.matmul(po[:], wf[:, 3, ki, :], xp[ki][:, 2:TL + 2],
                                         start=False, stop=False); n += 1
                        nc.tensor.matmul(po[:], wf[:, 4, ki, :], sko[ki][:],
                                         start=False, stop=(n == 11)); n += 1
                    # blocked layout: [even 512 | odd 512]
                    ft = fpool.tile([128, 1024], bf16, tag="fused", name="ft")
                    nc.scalar.activation(ft[:, 0:512], pe[:], AF.Identity, bias=bias_col(od))
                    nc.scalar.activation(ft[:, 512:1024], po[:], AF.Identity, bias=bias_col(od))
                    fused.append(ft)
                    sq = sqpool.tile([128, 1024], bf16, tag="sq", name="sq")
                    nc.scalar.activation(sq[:], ft[:], AF.Square)
                    sqs.append(sq)

                # ---------- rstd ----------
                rst = spool.tile([1, T], bf16, tag="rstd", name="rst")
                lnt = spool.tile([1, T], f32, tag="lnt", name="lnt")
                for ch in range(2):
                    pss = psm.tile([1, 512], f32, tag="pss", name="pss")
                    for ki in range(4):
                        nc.tensor.matmul(pss[:], onesb[:], sqs[ki][:, ch * 512:(ch + 1) * 512],
                                         start=(ki == 0), stop=(ki == 3))
                    nc.scalar.activation(lnt[:, ch * 512:(ch + 1) * 512], pss[:],
                                         AF.Ln, bias=bias[0:1, 44:45], scale=1.0 / D)
                nc.scalar.activation(rst[:], lnt[:], AF.Exp, scale=-0.5)
                d_rstd = dpool.tile([1, T], bf16, tag="drstd", name="d_rstd")
                nc.sync.dma_start(d_rstd[:], rst[:])
                rrep = rrpool.tile([128, T], bf16, tag="rrep", name="rrep")
                nc.sync.dma_start(rrep[:], d_rstd[:].to_broadcast((128, T)))

                # ---------- LN scale (blocked in -> interleaved natural-t out) ----------
                fn = []
                for od in range(4):
                    t_ = fnpool.tile([128, T], bf16, tag="fn", name="fn")
                    nc.vector.tensor_tensor(
                        out=t_[:].rearrange("p (a b) -> p b a", b=2),
                        in0=fused[od][:], in1=rrep[:], op=OP.mult)
                    fn.append(t_)

                # ---------- in_proj (x half) + conv + silu ----------
                xss = []
                for mt in range(8):
                    wi = wpool.tile([128, 4, 128], bf16, tag="wi", name="wi")
                    nc.sync.dma_start(wi[:], d_win[:, mt])
                    xt = xinpool.tile([128, T + 3], bf16, tag="xin", name="xt")
                    nc.vector.memset(xt[:, 0:3], 0.0)
                    for ch in range(2):
                        pm = pmm.tile([128, 512], f32, tag="pm", name="pm")
                        for ki in range(4):
                            nc.tensor.matmul(pm[:], wi[:, ki, :],
                                             fn[ki][:, ch * 512:(ch + 1) * 512],
                                             start=(ki == 0), stop=(ki == 3))
                        nc.scalar.activation(
                            xt[:, 3 + ch * 512:3 + (ch + 1) * 512], pm[:],
                            AF.Identity, bias=bias_col(4 + mt))
                    # depthwise conv k=4 + silu
                    wc = wpool.tile([128, 4, 128], bf16, tag="wc", name="wc")
                    nc.sync.dma_start(wc[:], d_wcv[:, mt])
                    xst = bigpool.tile([128, T], bf16, tag="xs", name="xst")
                    for ch in range(2):
                        pm = pmm.tile([128, 512], f32, tag="pm", name="pmc")
                        for k in range(4):
                            nc.tensor.matmul(pm[:], wc[:, k, :],
                                             xt[:, k + ch * 512:k + (ch + 1) * 512],
                                             start=(k == 0), stop=(k == 3))
                        nc.scalar.activation(xst[:, ch * 512:(ch + 1) * 512], pm[:],
                                             AF.Silu, bias=bias_col(20 + mt))
                    xss.append(xst)

                # ---------- xproj ----------
                xdb = xdpool.tile([64, T], bf16, tag="xdbl", name="xdb")
                for ch in range(2):
                    pm = pmm.tile([64, 512], f32, tag="pm", name="pmx")
                    for ki in range(8):
                        nc.tensor.matmul(pm[:], wxp[:, ki, :],
                                         xss[ki][:, ch * 512:(ch + 1) * 512],
                                         start=(ki == 0), stop=(ki == 7))
                    nc.scalar.activation(xdb[:, ch * 512:(ch + 1) * 512], pm[:], AF.Copy)
                d_bc = dpool.tile([32, T], bf16, tag="dbc", name="d_bc")
                nc.sync.dma_start(d_bc[:], xdb[32:64, :])

                # ---------- dt = softplus(dt_w @ dtr + dt_b) ----------
                dts = []
                for dt_ in range(8):
                    msl = slice(dt_ * 128, (dt_ + 1) * 128)
                    dtt = bigpool.tile([128, T], bf16, tag="dt", name="dtt")
                    for ch in range(2):
                        pm = pmm.tile([128, 512], f32, tag="pm", name="pmd")
                        nc.tensor.matmul(pm[:], wdt[0:DTR, msl],
                                         xdb[0:DTR, ch * 512:(ch + 1) * 512],
                                         start=True, stop=True)
                        pe2 = psm.tile([128, 512], f32, tag="pexp", name="pe2")
                        nc.scalar.activation(pe2[:], pm[:], AF.Exp,
                                             bias=bias_col(28 + dt_))
                        nc.scalar.activation(dtt[:, ch * 512:(ch + 1) * 512], pe2[:],
                                             AF.Ln, bias=1.0)
                    dts.append(dtt)

                # ---------- scan block (two s-groups of 8) ----------
                ys = [None] * 8
                for sg in range(2):
                    brep, crep = {}, {}
                    for s8 in range(8):
                        s = sg * 8 + s8
                        bt_ = reppool.tile([128, T], bf16, tag="brep", name="brt")
                        nc.sync.dma_start(bt_[:], d_bc[s:s + 1, :].to_broadcast((128, T)))
                        brep[s] = bt_
                        ct_ = reppool.tile([128, T], bf16, tag="crep", name="crt")
                        nc.sync.dma_start(ct_[:], d_bc[S + s:S + s + 1, :].to_broadcast((128, T)))
                        crep[s] = ct_
                    for dt_ in range(8):
                        ut = upool.tile([128, T], bf16, tag="u", name="ut")
                        nc.vector.tensor_tensor(out=ut[:], in0=dts[dt_][:],
                                                in1=xss[dt_][:], op=OP.mult)
                        pys = [pym.tile([128, 512], f32, tag="py0", name="py0"),
                               pym.tile([128, 512], f32, tag="py1", name="py1")]
                        for s8 in range(8):
                            s = sg * 8 + s8
                            at = apool.tile([128, T], DEC_DT, tag="a", name="at")
                            nc.scalar.activation(at[:], dts[dt_][:], AF.Exp,
                                                 scale=float(A[s]))
                            eng_bt = nc.gpsimd if s8 < BT_ON_GPS else nc.vector
                            eng_pr = nc.gpsimd if s8 < PROD_ON_GPS else nc.vector
                            btt = btpool.tile([128, T], bf16, tag="bt", name="btt")
                            eng_bt.tensor_tensor(out=btt[:], in0=ut[:],
                                                 in1=brep[s][:], op=OP.mult)
                            ht = hpool.tile([128, T], bf16, tag="h", name="ht")
                            nc.vector.tensor_tensor_scan(
                                out=ht[:], data0=at[:], data1=btt[:],
                                initial=0.0, op0=OP.mult, op1=OP.add)
                            pt = prpool.tile([128, T], bf16, tag="prod", name="pt")
                            eng_pr.tensor_tensor(out=pt[:], in0=ht[:],
                                                 in1=crep[s][:], op=OP.mult)
                            for ch in range(2):
                                csl = slice(ch * 512, (ch + 1) * 512)
                                nc.tensor.matmul(pys[ch][:], idt[:], pt[:, csl],
                                                 start=(s8 == 0), stop=(s8 == 7))
                        if sg == 0:
                            yt = ypool.tile([128, T], bf16, tag="ysb", name="yt")
                            ys[dt_] = yt
                        else:
                            yt = ys[dt_]
                        for ch in range(2):
                            csl = slice(ch * 512, (ch + 1) * 512)
                            if sg == 0:
                                nc.vector.scalar_tensor_tensor(
                                    out=yt[:, csl], in0=xss[dt_][:, csl],
                                    scalar=bias_col(36 + dt_), in1=pys[ch][:],
                                    op0=OP.mult, op1=OP.add)
                            else:
                                nc.vector.scalar_tensor_tensor(
                                    out=yt[:, csl], in0=pys[ch][:], scalar=1.0,
                                    in1=yt[:, csl], op0=OP.mult, op1=OP.add)

                # ---------- z half of in_proj + gating (in-place on ys) ----------
                for zd in range(8):
                    wi = wpool.tile([128, 4, 128], bf16, tag="wi", name="wiz")
                    nc.sync.dma_start(wi[:], d_win[:, 8 + zd])
                    zt = zpool.tile([128, T], bf16, tag="z", name="zt")
                    for ch in range(2):
                        pm = pmm.tile([128, 512], f32, tag="pm", name="pmz")
                        for ki in range(4):
                            nc.tensor.matmul(pm[:], wi[:, ki, :],
                                             fn[ki][:, ch * 512:(ch + 1) * 512],
                                             start=(ki == 0), stop=(ki == 3))
                        nc.scalar.activation(zt[:, ch * 512:(ch + 1) * 512], pm[:],
                                             AF.Silu, bias=bias_col(12 + zd))
                    nc.vector.tensor_tensor(out=ys[zd][:], in0=ys[zd][:], in1=zt[:],
                                            op=OP.mult)

                # ---------- out_proj ----------
                for od in range(4):
                    wo = wpool.tile([128, 8, 128], bf16, tag="wo", name="wo")
                    nc.sync.dma_start(wo[:], d_wout[:, od])
                    for ch in range(2):
                        pm = pmm.tile([128, 512], f32, tag="pm", name="pmo")
                        for ki in range(8):
                            nc.tensor.matmul(pm[:], wo[:, ki, :],
                                             ys[ki][:, ch * 512:(ch + 1) * 512],
                                             start=(ki == 0), stop=(ki == 7))
                        ot = opool.tile([128, 512], f32, tag="out", name="ot")
                        nc.scalar.activation(ot[:], pm[:], AF.Copy)
                        nc.sync.dma_start(
                            d_out[b, od * 128:(od + 1) * 128, ch * 512:(ch + 1) * 512],
                            ot[:])
    nc.finalize()
    return nc


TRACE = False


def kernel(**inputs):
    global _BUILT
    per_core, weights, A = _host_prep(inputs)
    if _BUILT is None:
        _BUILT = _build(A)
    nc = _BUILT
    from concourse.bass_utils import run_bass_kernel_spmd
    in_maps = []
    for c in range(NCORES):
        m = dict(weights)
        m.update(per_core[c])
        in_maps.append(m)
    res = run_bass_kernel_spmd(nc, in_maps, core_ids=list(range(NCORES)),
                               trace=TRACE)
    import kernel as _self
    _self.LAST_RESULT = res
    out = np.empty((BTOT, T, D), np.float32)
    for c in range(NCORES):
        outT = res.results[c]["outT"]          # (BL, D, T)
        out[c * BL:(c + 1) * BL] = outT.transpose(0, 2, 1)
    return out
